# revision 18
# baseline (speedup 1.0000x reference)
"""Trainium2 Bass kernel for nn_DeconvNet1 (GAT encoder/decoder, 8-core SPMD).

Model (see reference): 2-layer GAT autoencoder over a 4096-node graph.
  encoder: H = elu(multihead_GAT(x, adj)); Z = H @ enc2_W + b
  decoder: Hd = elu(multihead_GAT([Z, se], adj)); recon = Hd @ dec2_W + b
Attention per head:  h = x @ W + b;  f1 = h@v0; f2 = h@v1
  logits = adj * (f1 + f2^T); u = sigmoid(logits) - 0.5
  p = masked_softmax_rows(u);  out = p @ h

Sharding: nodes (rows of adj and activations) split 512/core across the 8
NeuronCores.  Each core keeps the full column dimension of its adj row-block
(transposed once on-chip via PE into adjT [4096j x 512m], bf16 exact 0/1),
so the row-softmax and attn@h matmul are local.  h / f2 of all nodes are
exchanged with two small DRAM AllGathers per GAT layer.

Math used on-chip (equivalent to the reference up to fp rounding):
  sigmoid(z) - 0.5 = 0.5*tanh(z/2)  -> tanh and exp share one ACT table set
  softmax needs no row-max (arguments bounded in (-0.5, 0.5)):
     w = adjT * exp(0.5*tanh(z/2));  p = w / sum_j(w)
  The per-row normalization commutes with the matmul:
     out.T = (h.T @ w) * (1/den) broadcast.

Precision: all matmuls are fp32 except the big attention matmuls (w.T @ h and
the den row-sum), which run as fp32r (TF32-like, 1 cycle/row instead of 4).
"""

import os
import sys

import numpy as np

for _p in ("/opt/trn_rl_repo", "/root/.axon_site/_ro/trn_rl_repo"):
    if os.path.isdir(_p) and _p not in sys.path:
        sys.path.insert(0, _p)
        break

from contextlib import ExitStack

import concourse.bacc as bacc
import concourse.bass as bass
import concourse.mybir as mybir
import concourse.tile as tile
from concourse.bass_utils import run_bass_kernel_spmd
from concourse.masks import make_identity

# ---- problem dims (hardcoded from the reference) ----
N, G, H1, H2 = 4096, 1024, 512, 64
NH, CH, SE = 4, 128, 16
HD = H2 + SE  # 80, decoder GAT input dim
NCORES = 8
NL = N // NCORES  # 512 rows per core
MT = NL // 128  # 4 local m-tiles
JT = N // 128  # 32 j-tiles
GJ = 4  # j-tiles per attention z-group
NGRP = JT // GJ  # 8 groups per head
KT = G // 128  # 8 k-chunks for enc1

F32 = mybir.dt.float32
F32R = mybir.dt.float32r
BF16 = mybir.dt.bfloat16
AF = mybir.ActivationFunctionType
ALU = mybir.AluOpType

# which z-groups' mask-multiplies run on GPSIMD instead of DVE (load balance)
GP_MASK_GROUPS = frozenset({5, 6, 7})


def _attention_layer(nc, pools, hfp, f2c, fT_sb, adjT, out_tiles, gat):
    """One 4-head GAT attention pass.

    hfp:  SBUF f32r [128, JT, H1]  gathered h of all nodes ([j, c] layout)
    f2c:  SBUF [128, JT] x NH      per-head f2 columns (partition j)
    fT_sb: SBUF [8, NL]            local f rows (f1 = rows 0..3)
    adjT: SBUF bf16 [128, JT, NL]
    out_tiles: list of NH SBUF tiles [128, NL] receiving elu(p @ h).T
    """
    work, psatt, psA = pools["work"], pools["psatt"], pools["psA"]
    ones_row, ones_colr = pools["ones_row"], pools["ones_colr"]
    sel = pools["sel"]

    for h in range(NH):
        # f1bc[p, m] = fT_sb[h, m] for all p (selector-row matmul broadcast)
        f1bc = work.tile([128, NL], F32, tag="f1bc", name=f"f1bc_{gat}_{h}")
        f1ps = psA.tile([128, NL], F32, tag="tp", name=f"f1ps_{gat}_{h}")
        nc.tensor.matmul(f1ps, sel[h], fT_sb, start=True, stop=True)
        nc.vector.tensor_copy(f1bc, f1ps)

        outT = psatt.tile([128, NL], F32, tag="outT", name=f"outT_{gat}_{h}")
        den = psatt.tile([1, NL], F32, tag="den", name=f"den_{gat}_{h}")

        for g in range(NGRP):
            z = work.tile([128, GJ, NL], F32, tag="zbuf", name=f"z_{gat}_{h}_{g}")
            for i in range(GJ):
                jt = g * GJ + i
                nc.vector.tensor_scalar_add(z[:, i, :], f1bc, f2c[h][:, jt : jt + 1])
            # tanh(z/2), then exp(0.5*tanh) == exp(sigmoid(z) - 0.5)
            nc.scalar.activation(z, z, AF.Tanh, scale=0.5)
            nc.scalar.activation(z, z, AF.Exp, scale=0.5)
            # mask by adjacency (0/1) -> attention weights w (fp32r for PE)
            w = work.tile([128, GJ, NL], F32R, tag="wbuf", name=f"w_{gat}_{h}_{g}")
            adj_sl = adjT[:, g * GJ : (g + 1) * GJ, :]
            if g in GP_MASK_GROUPS:
                nc.gpsimd.tensor_mul(w, z, adj_sl)
            else:
                nc.vector.tensor_mul(w, z, adj_sl)
            for i in range(GJ):
                jt = g * GJ + i
                start, stop = jt == 0, jt == JT - 1
                lhs = hfp[:, jt, h * CH : (h + 1) * CH]
                nc.tensor.matmul(outT, lhs, w[:, i, :], start=start, stop=stop)
                nc.tensor.matmul(den, ones_colr, w[:, i, :], start=start, stop=stop)

        # normalize rows: out = outT / den, then elu
        rd = work.tile([1, NL], F32, tag="rd", name=f"rd_{gat}_{h}")
        nc.vector.reciprocal(rd, den)
        rdps = psA.tile([128, NL], F32, tag="tp", name=f"rdps_{gat}_{h}")
        nc.tensor.matmul(rdps, ones_row[0:1, 0:128], rd, start=True, stop=True)
        rdbc = work.tile([128, NL], F32, tag="rdbc", name=f"rdbc_{gat}_{h}")
        nc.vector.tensor_copy(rdbc, rdps)
        ot = out_tiles[h]
        nc.vector.tensor_mul(ot, outT, rdbc)
        # elu(x) = max(x,0) + exp(min(x,0)) - 1
        scr = work.tile([128, NL], F32, tag="scr", name=f"scr_{gat}_{h}")
        nc.vector.tensor_scalar_min(scr, ot, 0.0)
        nc.scalar.activation(scr, scr, AF.Exp)
        nc.vector.tensor_scalar_max(ot, ot, 0.0)
        nc.vector.tensor_add(ot, ot, scr)
        nc.vector.tensor_scalar_add(ot, ot, -1.0)


def _build_program():
    nc = bacc.Bacc("TRN2", num_devices=NCORES)

    # ---- per-core I/O ----
    adj = nc.declare_dram_parameter("adj", [NL, N], F32, isOutput=False)
    x = nc.declare_dram_parameter("x", [NL, G], F32, isOutput=False)
    seT = nc.declare_dram_parameter("seT", [SE, NL], F32, isOutput=False)
    enc1_W = nc.declare_dram_parameter("enc1_W", [NH, G, CH], F32, isOutput=False)
    enc1_b = nc.declare_dram_parameter("enc1_b", [NH, CH], F32, isOutput=False)
    enc2_W = nc.declare_dram_parameter("enc2_W", [H1, H2], F32, isOutput=False)
    enc2_b = nc.declare_dram_parameter("enc2_b", [H2], F32, isOutput=False)
    dec1_W = nc.declare_dram_parameter("dec1_W", [NH, HD, CH], F32, isOutput=False)
    dec1_b = nc.declare_dram_parameter("dec1_b", [NH, CH], F32, isOutput=False)
    dec2_W = nc.declare_dram_parameter("dec2_W", [H1, G], F32, isOutput=False)
    dec2_b = nc.declare_dram_parameter("dec2_b", [G], F32, isOutput=False)
    encWf = nc.declare_dram_parameter("encWf", [G, 2 * NH], F32, isOutput=False)
    encfb = nc.declare_dram_parameter("encfb", [1, 2 * NH], F32, isOutput=False)
    decWf = nc.declare_dram_parameter("decWf", [HD, 2 * NH], F32, isOutput=False)
    decfb = nc.declare_dram_parameter("decfb", [1, 2 * NH], F32, isOutput=False)
    recon = nc.declare_dram_parameter("recon", [NL, G], F32, isOutput=True)
    Zout = nc.declare_dram_parameter("Z", [NL, H2], F32, isOutput=True)

    groups = [list(range(NCORES))]
    dma = nc.sync.dma_start

    with tile.TileContext(nc) as tc, ExitStack() as st:
        const = st.enter_context(tc.tile_pool(name="const", bufs=1))
        big = st.enter_context(tc.tile_pool(name="big", bufs=1))
        dram = st.enter_context(tc.tile_pool(name="dram", bufs=1, space="DRAM"))
        psA = st.enter_context(tc.tile_pool(name="psA", bufs=2, space="PSUM"))
        psatt = st.enter_context(tc.tile_pool(name="psatt", bufs=2, space="PSUM"))
        psF = st.enter_context(tc.tile_pool(name="psF", bufs=1, space="PSUM"))

        # ---------------- constants ----------------
        ident = const.tile([128, 128], F32)
        make_identity(nc, ident)
        ones_row = const.tile([1, NL], F32)
        nc.gpsimd.memset(ones_row, 1.0)
        ones_col = const.tile([128, 1], F32)
        nc.gpsimd.memset(ones_col, 1.0)
        ones_colr = const.tile([128, 1], F32R)
        nc.vector.tensor_copy(ones_colr, ones_col)
        # sel[h]: [8, 128] with row h = ones -> selector for f1 broadcast
        sel = []
        for h in range(NH):
            s = const.tile([8, 128], F32, name=f"sel_{h}")
            nc.gpsimd.memset(s, 0.0)
            nc.gpsimd.affine_select(
                out=s,
                in_=s,
                compare_op=ALU.not_equal,
                fill=1.0,
                base=-h,
                pattern=[[0, 128]],
                channel_multiplier=1,
            )
            sel.append(s)

        encWf_sb = const.tile([128, KT, 2 * NH], F32)
        for kt in range(KT):
            dma(out=encWf_sb[:, kt, :], in_=encWf[kt * 128 : (kt + 1) * 128, :])
        encfb_sb = const.tile([1, 2 * NH], F32)
        dma(out=encfb_sb, in_=encfb[:, :])
        encb_row = const.tile([1, NH * CH], F32)
        for h in range(NH):
            dma(out=encb_row[0:1, h * CH : (h + 1) * CH], in_=enc1_b[h : h + 1, :])
        enc2W_sb = const.tile([128, 4, H2], F32)
        for kc in range(4):
            dma(out=enc2W_sb[:, kc, :], in_=enc2_W[kc * 128 : (kc + 1) * 128, :])
        enc2b_sb = const.tile([1, H2], F32)
        dma(out=enc2b_sb, in_=enc2_b[None, :])
        dec1Wb = const.tile([HD, NH * CH], F32)
        dec1b_row = const.tile([1, NH * CH], F32)
        for h in range(NH):
            dma(out=dec1Wb[:, h * CH : (h + 1) * CH], in_=dec1_W[h, :, :])
            dma(out=dec1b_row[0:1, h * CH : (h + 1) * CH], in_=dec1_b[h : h + 1, :])
        decWf_sb = const.tile([HD, 2 * NH], F32)
        dma(out=decWf_sb, in_=decWf[:, :])
        decfb_sb = const.tile([1, 2 * NH], F32)
        dma(out=decfb_sb, in_=decfb[:, :])
        dec2b_sb = const.tile([1, G], F32)
        dma(out=dec2b_sb, in_=dec2_b[None, :])

        adjT = big.tile([128, JT, NL], BF16)

        # ---- DRAM bounce buffers for the collectives ----
        gin_h = {}
        gout_h = {}
        gin_f = {}
        gout_f = {}
        for gat in ("enc", "dec"):
            gin_h[gat] = dram.tile([NL, H1], F32, name=f"gin_h_{gat}")
            gout_h[gat] = dram.tile(
                [N, H1], F32, addr_space="Shared", name=f"gout_h_{gat}"
            )
            gin_f[gat] = dram.tile([NH, NL], F32, name=f"gin_f_{gat}")
            gout_f[gat] = dram.tile(
                [NH * NCORES, NL], F32, addr_space="Shared", name=f"gout_f_{gat}"
            )

        # ================= P0: load + transpose adj and x =================
        with tc.tile_pool(name="loadp", bufs=1) as loadp:
            # x -> xT [k, m]
            xT = loadp.tile([128, KT, NL], F32)
            with tc.tile_pool(name="xload", bufs=2) as xload:
                for mt in range(MT):
                    xt_ = xload.tile([128, G], F32, tag="x", name=f"x_{mt}")
                    dma(out=xt_, in_=x[mt * 128 : (mt + 1) * 128, :])
                    for kt in range(KT):
                        tp = psA.tile([128, NL], F32, tag="tp", name=f"xtp_{mt}_{kt}")
                        nc.tensor.transpose(
                            tp[:, mt * 128 : (mt + 1) * 128],
                            xt_[:, kt * 128 : (kt + 1) * 128],
                            ident,
                        )
                        nc.vector.tensor_copy(
                            xT[:, kt, mt * 128 : (mt + 1) * 128],
                            tp[:, mt * 128 : (mt + 1) * 128],
                        )

            # adj -> adjT (bf16) ; psum batches of 4 j-tiles per copy
            with tc.tile_pool(name="adjload", bufs=2) as adjload:
                for mt in range(MT):
                    at_ = adjload.tile([128, N], F32, tag="adj", name=f"adj_{mt}")
                    dma(out=at_, in_=adj[mt * 128 : (mt + 1) * 128, :])
                    for jq in range(JT // 4):
                        tp = psA.tile([128, NL], F32, tag="tp", name=f"atp_{mt}_{jq}")
                        for i in range(4):
                            jt = jq * 4 + i
                            nc.tensor.transpose(
                                tp[:, i * 128 : (i + 1) * 128],
                                at_[:, jt * 128 : (jt + 1) * 128],
                                ident,
                            )
                        dst = adjT[:, jq * 4 : (jq + 1) * 4, mt * 128 : (mt + 1) * 128]
                        src = tp.rearrange("p (i q) -> p i q", i=4)
                        nc.vector.tensor_copy(dst, src)

            # ============ P1: enc1 h + fT, ship to allgather ============
            encW_sb = loadp.tile([128, KT, NH * CH], F32)
            for kt in range(KT):
                for h in range(NH):
                    dma(
                        out=encW_sb[:, kt, h * CH : (h + 1) * CH],
                        in_=enc1_W[h, kt * 128 : (kt + 1) * 128, :],
                    )

            fT_ps = psF.tile([8, NL], F32, tag="fT", name="fT_enc_ps")
            for kt in range(KT):
                nc.tensor.matmul(
                    fT_ps,
                    encWf_sb[:, kt, :],
                    xT[:, kt, :],
                    start=(kt == 0),
                    stop=False,
                )
            nc.tensor.matmul(fT_ps, encfb_sb, ones_row, start=False, stop=True)
            fT_enc = const.tile([8, NL], F32, name="fT_enc")
            nc.vector.tensor_copy(fT_enc, fT_ps)
            dma(out=gin_f["enc"], in_=fT_enc[4:8, :])

            for mt in range(MT):
                hps = psA.tile([128, NH * CH], F32, tag="tp", name=f"hps_{mt}")
                for kt in range(KT):
                    nc.tensor.matmul(
                        hps,
                        xT[:, kt, mt * 128 : (mt + 1) * 128],
                        encW_sb[:, kt, :],
                        start=(kt == 0),
                        stop=False,
                    )
                nc.tensor.matmul(
                    hps, ones_row[0:1, 0:128], encb_row, start=False, stop=True
                )
                hsb = loadp.tile(
                    [128, NH * CH], F32, tag="hsb", bufs=2, name=f"hsb_{mt}"
                )
                nc.vector.tensor_copy(hsb, hps)
                dma(out=gin_h["enc"][mt * 128 : (mt + 1) * 128, :], in_=hsb)

        # ================= P2: allgather (enc) =================
        nc.gpsimd.collective_compute(
            "AllGather",
            ALU.bypass,
            replica_groups=groups,
            ins=[gin_h["enc"].opt()],
            outs=[gout_h["enc"].opt()],
        )
        nc.gpsimd.collective_compute(
            "AllGather",
            ALU.bypass,
            replica_groups=groups,
            ins=[gin_f["enc"].opt()],
            outs=[gout_f["enc"].opt()],
        )

        hfp = big.tile([128, JT, H1], F32R, tag="hfp", name="hfp_enc")
        src = gout_h["enc"][:].rearrange("(jt p) c -> p jt c", p=128).bitcast(F32R)
        dma(out=hfp, in_=src)
        f2c_enc = []
        for h in range(NH):
            t = const.tile([128, JT], F32, name=f"f2c_enc_{h}")
            for cc in range(NCORES):
                fsrc = gout_f["enc"][cc * NH + h, :].rearrange(
                    "(jtl p) -> p jtl", p=128
                )
                dma(out=t[:, cc * MT : (cc + 1) * MT], in_=fsrc)
            f2c_enc.append(t)

        # ================= P3: encoder attention =================
        work = st.enter_context(tc.tile_pool(name="work", bufs=2))
        HT_pool = st.enter_context(tc.tile_pool(name="HT", bufs=NH))
        pools = {
            "work": work,
            "psatt": psatt,
            "psA": psA,
            "ones_row": ones_row,
            "ones_colr": ones_colr,
            "sel": sel,
        }

        HT = [HT_pool.tile([128, NL], F32, tag="HT", name=f"HT_{h}") for h in range(NH)]
        _attention_layer(nc, pools, hfp, f2c_enc, fT_enc, adjT, HT, "enc")

        # ================= P4: Z = H @ enc2_W + b =================
        HdT = const.tile([HD, NL], F32, name="HdT")
        dma(out=HdT[H2:HD, :], in_=seT[:, :])
        for mt in range(MT):
            zfull = psA.tile([128, NL], F32, tag="tp", name=f"zps_{mt}")
            zps = zfull[:, 0:H2]
            for kc in range(4):
                nc.tensor.matmul(
                    zps,
                    HT[kc][:, mt * 128 : (mt + 1) * 128],
                    enc2W_sb[:, kc, :],
                    start=(kc == 0),
                    stop=False,
                )
            nc.tensor.matmul(
                zps, ones_row[0:1, 0:128], enc2b_sb, start=False, stop=True
            )
            zsb = work.tile([128, H2], F32, tag="zsb", name=f"zsb_{mt}")
            nc.vector.tensor_copy(zsb, zps)
            dma(out=Zout[mt * 128 : (mt + 1) * 128, :], in_=zsb)
            ztp = psA.tile([128, NL], F32, tag="tp", name=f"ztp_{mt}")
            nc.tensor.transpose(ztp[0:H2, 0:128], zsb, ident)
            nc.vector.tensor_copy(
                HdT[0:H2, mt * 128 : (mt + 1) * 128], ztp[0:H2, 0:128]
            )

        # ================= P5: dec1 + allgather (dec) =================
        fTd_ps = psF.tile([8, NL], F32, tag="fT", name="fT_dec_ps")
        nc.tensor.matmul(fTd_ps, decWf_sb, HdT, start=True, stop=False)
        nc.tensor.matmul(fTd_ps, decfb_sb, ones_row, start=False, stop=True)
        fT_dec = const.tile([8, NL], F32, name="fT_dec")
        nc.vector.tensor_copy(fT_dec, fTd_ps)
        dma(out=gin_f["dec"], in_=fT_dec[4:8, :])

        for mt in range(MT):
            hps = psA.tile([128, NH * CH], F32, tag="tp", name=f"hdps_{mt}")
            nc.tensor.matmul(
                hps,
                HdT[:, mt * 128 : (mt + 1) * 128],
                dec1Wb,
                start=True,
                stop=False,
            )
            nc.tensor.matmul(
                hps, ones_row[0:1, 0:128], dec1b_row, start=False, stop=True
            )
            hsbd = work.tile([128, NH * CH], F32, tag="hsbd", name=f"hsbd_{mt}")
            nc.vector.tensor_copy(hsbd, hps)
            dma(out=gin_h["dec"][mt * 128 : (mt + 1) * 128, :], in_=hsbd)

        nc.gpsimd.collective_compute(
            "AllGather",
            ALU.bypass,
            replica_groups=groups,
            ins=[gin_h["dec"].opt()],
            outs=[gout_h["dec"].opt()],
        )
        nc.gpsimd.collective_compute(
            "AllGather",
            ALU.bypass,
            replica_groups=groups,
            ins=[gin_f["dec"].opt()],
            outs=[gout_f["dec"].opt()],
        )

        hfp_d = big.tile([128, JT, H1], F32R, tag="hfp", name="hfp_dec")
        src = gout_h["dec"][:].rearrange("(jt p) c -> p jt c", p=128).bitcast(F32R)
        dma(out=hfp_d, in_=src)
        f2c_dec = []
        for h in range(NH):
            t = const.tile([128, JT], F32, name=f"f2c_dec_{h}")
            for cc in range(NCORES):
                fsrc = gout_f["dec"][cc * NH + h, :].rearrange(
                    "(jtl p) -> p jtl", p=128
                )
                dma(out=t[:, cc * MT : (cc + 1) * MT], in_=fsrc)
            f2c_dec.append(t)

        # ================= P6: decoder attention =================
        HdO = [
            HT_pool.tile([128, NL], F32, tag="HT", name=f"HdO_{h}") for h in range(NH)
        ]
        _attention_layer(nc, pools, hfp_d, f2c_dec, fT_dec, adjT, HdO, "dec")

        # ================= P7: recon = Hd @ dec2_W + b =================
        with tc.tile_pool(name="dec2p", bufs=1) as dec2p:
            dec2W_sb = dec2p.tile([128, 4, G], F32)
            for kc in range(4):
                dma(out=dec2W_sb[:, kc, :], in_=dec2_W[kc * 128 : (kc + 1) * 128, :])
            for mt in range(MT):
                for half in range(2):
                    rps = psA.tile([128, 512], F32, tag="tp", name=f"rps_{mt}_{half}")
                    for kc in range(4):
                        nc.tensor.matmul(
                            rps,
                            HdO[kc][:, mt * 128 : (mt + 1) * 128],
                            dec2W_sb[:, kc, half * 512 : (half + 1) * 512],
                            start=(kc == 0),
                            stop=False,
                        )
                    nc.tensor.matmul(
                        rps,
                        ones_row[0:1, 0:128],
                        dec2b_sb[0:1, half * 512 : (half + 1) * 512],
                        start=False,
                        stop=True,
                    )
                    rsb = dec2p.tile(
                        [128, 512], F32, tag="rsb", bufs=2, name=f"rsb_{mt}_{half}"
                    )
                    nc.vector.tensor_copy(rsb, rps)
                    dma(
                        out=recon[
                            mt * 128 : (mt + 1) * 128, half * 512 : (half + 1) * 512
                        ],
                        in_=rsb,
                    )

    nc.finalize()
    return nc


_NC = None


def _get_program():
    global _NC
    if _NC is None:
        _NC = _build_program()
    return _NC


def _prep_in_maps(inputs):
    g = lambda k: np.ascontiguousarray(np.asarray(inputs[k]), dtype=np.float32)
    adj = g("adj_matrix")
    x = g("node_feats")
    emb = g("emb")
    lab = np.asarray(inputs["slice_label"]).astype(np.int64)
    se = emb[lab]  # [N, SE]

    enc1_W, enc1_b = g("enc1_W"), g("enc1_b")
    enc1_v0, enc1_v1 = g("enc1_v0"), g("enc1_v1")
    dec1_W, dec1_b = g("dec1_W"), g("dec1_b")
    dec1_v0, dec1_v1 = g("dec1_v0"), g("dec1_v1")

    encWf = np.zeros((G, 2 * NH), np.float32)
    encfb = np.zeros((1, 2 * NH), np.float32)
    decWf = np.zeros((HD, 2 * NH), np.float32)
    decfb = np.zeros((1, 2 * NH), np.float32)
    for h in range(NH):
        encWf[:, h] = enc1_W[h] @ enc1_v0[h]
        encWf[:, NH + h] = enc1_W[h] @ enc1_v1[h]
        encfb[0, h] = enc1_b[h] @ enc1_v0[h]
        encfb[0, NH + h] = enc1_b[h] @ enc1_v1[h]
        decWf[:, h] = dec1_W[h] @ dec1_v0[h]
        decWf[:, NH + h] = dec1_W[h] @ dec1_v1[h]
        decfb[0, h] = dec1_b[h] @ dec1_v0[h]
        decfb[0, NH + h] = dec1_b[h] @ dec1_v1[h]

    shared = {
        "enc1_W": enc1_W,
        "enc1_b": enc1_b,
        "enc2_W": g("enc2_W"),
        "enc2_b": g("enc2_b"),
        "dec1_W": dec1_W,
        "dec1_b": dec1_b,
        "dec2_W": g("dec2_W"),
        "dec2_b": g("dec2_b"),
        "encWf": encWf,
        "encfb": encfb,
        "decWf": decWf,
        "decfb": decfb,
    }
    in_maps = []
    for c in range(NCORES):
        sl = slice(c * NL, (c + 1) * NL)
        m = dict(shared)
        m["adj"] = adj[sl]
        m["x"] = x[sl]
        m["seT"] = np.ascontiguousarray(se[sl].T)
        in_maps.append(m)
    return in_maps


def kernel(**inputs):
    nc = _get_program()
    in_maps = _prep_in_maps(inputs)
    res = run_bass_kernel_spmd(nc, in_maps, list(range(NCORES)))
    recon = np.concatenate([res.results[c]["recon"] for c in range(NCORES)], axis=0)
    Z = np.concatenate([res.results[c]["Z"] for c in range(NCORES)], axis=0)
    return recon, Z
